# revision 17
# baseline (speedup 1.0000x reference)
"""MultiHeadAttention forward on 8 TRN2 NeuronCores (Bass/Tile) — span-major bf16.

Problem: x[4,2048,1024], per-head Wq/Wk/Wv [16,1024,64], out proj Wp[1024,1024]+bp.
    q = einsum('btc,hcd->bhtd', x, Wq); wei = softmax(causal(q k^T / 32)); o = wei v
    y = concat_heads(o) @ Wp + bp

Sharding: core c <-> (batch b=c//2, head-group g=c%2, 8 heads each).  Work is
span-major: for each 512-token span s, the QKV projections for chunk s+1 are
emission-pumped between the attention units of span s so the PE stays busy
behind the scalar-engine exps.  Per span, the pair (2b, 2b+1) exchanges the
256-token quarter it does not own; core (b,g) projects tokens
[s*512+g*256, s*512+(g+1)*256) of every span.  The output projection for span
s-1 is pumped into span s's attention only from the third head-pair on, so it
can never head-of-line-block the PE queue on the pair exchange.

x arrives host-transposed as bf16 [C, T] (no on-chip transposes).  All
activations are bf16 (full PE rate); scores/outputs accumulate in fp32 PSUM.
Causal masking is additive (-1e5 triangle) on the score PSUM before exp;
fully-masked column ranges of diagonal tiles are skipped in the S matmul,
exp, and PV.  Softmax denominators come free from a ones-column in V;
reciprocal uses the fast approx DVE op.
"""
import numpy as np

B, T, C = 4, 2048, 1024
H, HS = 16, 64
HPC = 8          # heads per core
NCORES = 8
SP = 512         # span
QT = 256         # owned quarter per span

_CACHE = {}


def _build_nc():
    import concourse.bass as bass
    import concourse.mybir as mybir
    import concourse.tile as tile
    from concourse import bacc
    from concourse.bass import ds
    from concourse.masks import make_identity

    F32 = mybir.dt.float32
    BF16 = mybir.dt.bfloat16
    AF = mybir.ActivationFunctionType
    PAIRS = [[0, 1], [2, 3], [4, 5], [6, 7]]

    nc = bacc.Bacc("TRN2", target_bir_lowering=False, debug=False, num_devices=NCORES)

    xbT = nc.dram_tensor("xbT", [C, T], BF16, kind="ExternalInput").ap()
    wq = nc.dram_tensor("wq", [C, 512], BF16, kind="ExternalInput").ap()
    wk = nc.dram_tensor("wk", [C, 512], BF16, kind="ExternalInput").ap()
    wv = nc.dram_tensor("wv", [C, 512], BF16, kind="ExternalInput").ap()
    wpo = nc.dram_tensor("wpo", [512, C], BF16, kind="ExternalInput").ap()
    wpx = nc.dram_tensor("wpx", [512, C], BF16, kind="ExternalInput").ap()
    bpr = nc.dram_tensor("bpr", [1, C], BF16, kind="ExternalInput").ap()
    onesd = nc.dram_tensor("onesd", [128, 128], BF16, kind="ExternalInput").ap()
    y = nc.dram_tensor("y", [4, QT, C], F32, kind="ExternalOutput").ap()

    with tile.TileContext(nc) as tc:
        pid_g = nc.gpsimd.partition_id()
        g_sv = nc.gpsimd.snap(pid_g % 2, max_val=1)
        roff = nc.gpsimd.snap(128 - g_sv * 128, max_val=128)
        t_own = [nc.gpsimd.snap(s * SP + g_sv * QT, max_val=s * SP + QT)
                 for s in range(4)]
        t_ctr = [nc.gpsimd.snap(s * SP + QT - g_sv * QT, max_val=s * SP + QT)
                 for s in range(4)]

        with tc.tile_pool(name="consts", bufs=1) as consts, \
             tc.tile_pool(name="wpool", bufs=1) as wpool, \
             tc.tile_pool(name="acts", bufs=1) as acts, \
             tc.tile_pool(name="sb", bufs=1) as sb, \
             tc.tile_pool(name="ps", bufs=1, space="PSUM") as ps, \
             tc.tile_pool(name="ccd", bufs=1, space="DRAM") as ccd:

            # additive causal mask for the partial 128-col strip of diagonal
            # tiles: tri[p, c] = 0 if c >= p else -1e5
            tri = consts.tile([128, 128], F32)
            nc.gpsimd.memset(tri[:], 0.0)
            nc.gpsimd.affine_select(
                out=tri[:], in_=tri[:], compare_op=mybir.AluOpType.is_ge,
                fill=-1e5, base=0, pattern=[[1, 128]], channel_multiplier=-1)
            ones_sb = consts.tile([128, 128], BF16)
            nc.scalar.dma_start(out=ones_sb[:], in_=onesd[:])
            bp_sb = consts.tile([1, C], BF16)
            nc.scalar.dma_start(out=bp_sb[:], in_=bpr[:])

            wq_sb = wpool.tile([128, 8, 512], BF16)
            wk_sb = wpool.tile([128, 8, 512], BF16)
            wv_sb = wpool.tile([128, 8, 512], BF16)
            wpo_sb = wpool.tile([128, 4, C], BF16)
            wpx_sb = wpool.tile([128, 4, C], BF16)
            nc.scalar.dma_start(out=wq_sb[:], in_=wq.rearrange("(k p) n -> p k n", p=128))
            nc.scalar.dma_start(out=wk_sb[:], in_=wk.rearrange("(k p) n -> p k n", p=128))
            nc.scalar.dma_start(out=wv_sb[:], in_=wv.rearrange("(k p) n -> p k n", p=128))
            nc.scalar.dma_start(out=wpo_sb[:], in_=wpo.rearrange("(k p) n -> p k n", p=128))
            nc.scalar.dma_start(out=wpx_sb[:], in_=wpx.rearrange("(k p) n -> p k n", p=128))

            k_T = acts.tile([128, 4, T], BF16)      # [d(2 heads), hp, t]
            q_T = acts.tile([128, 4, T], BF16)
            attn_T = acts.tile([128, 4, T], BF16)
            v_aug = acts.tile([128, 16, 8 * 65], BF16)  # [t(128), t-tile, h*65+d]
            nc.vector.tensor_copy(
                v_aug[:].rearrange("p i (h e) -> p i h e", e=65)[:, :, :, 64:65],
                ones_sb[:, 0:128].rearrange("p (i h) -> p i h", h=8))

            def qkv_gen(s):
                """QKV projections for token chunk s."""
                xT = sb.tile([128, 8, 512], BF16, tag="xT", bufs=2, name=f"xT{s}")
                nc.sync.dma_start(
                    out=xT[:],
                    in_=xbT.rearrange("(k p) t -> p k t", p=128)[:, :, s * SP:(s + 1) * SP])
                for m in range(4):
                    yield
                    psq = ps.tile([128, 512], F32, tag="m", bufs=2,
                                  name=f"psq{s}{m}")
                    for cb in range(8):
                        nc.tensor.matmul(
                            psq[:], wq_sb[:, cb, m * 128:(m + 1) * 128],
                            xT[:, cb, :], start=(cb == 0), stop=(cb == 7))
                    nc.vector.tensor_copy(q_T[:, m, s * SP:(s + 1) * SP], psq[:])
                    yield
                    psk = ps.tile([128, 512], F32, tag="m", bufs=2,
                                  name=f"psk{s}{m}")
                    for cb in range(8):
                        nc.tensor.matmul(
                            psk[:], wk_sb[:, cb, m * 128:(m + 1) * 128],
                            xT[:, cb, :], start=(cb == 0), stop=(cb == 7))
                    nc.vector.tensor_copy(k_T[:, m, s * SP:(s + 1) * SP], psk[:])
                for i in range(4):
                    yield
                    ti = s * 4 + i
                    psv = ps.tile([128, 512], F32, tag="m", bufs=2,
                                  name=f"psv{s}{i}")
                    for cb in range(8):
                        nc.tensor.matmul(
                            psv[:], xT[:, cb, i * 128:(i + 1) * 128],
                            wv_sb[:, cb, :], start=(cb == 0), stop=(cb == 7))
                    nc.scalar.copy(
                        v_aug[:, ti, :].rearrange("p (h e) -> p h e", e=65)[:, :, 0:64],
                        psv[:].rearrange("p (h e) -> p h e", e=64))

            own_sb = {}
            rem_sb = {}

            def exchange(s):
                cc_in = ccd.tile([128, 4 * QT], BF16, tag="ccin", bufs=2,
                                 name=f"ccin{s}")
                cc_out = ccd.tile([256, 4 * QT], BF16, tag="ccout", bufs=2,
                                  name=f"ccout{s}")
                own = sb.tile([128, 4, QT], BF16, tag="own", bufs=2,
                              name=f"own{s}")
                rem = sb.tile([128, 4, QT], BF16, tag="rem", bufs=2,
                              name=f"rem{s}")
                nc.gpsimd.dma_start(
                    out=cc_in[:].rearrange("p (m t) -> p m t", t=QT),
                    in_=attn_T[:, :, ds(t_ctr[s], QT)])
                nc.gpsimd.collective_compute(
                    "AllGather", mybir.AluOpType.bypass,
                    ins=[cc_in.opt()], outs=[cc_out.opt()],
                    replica_groups=PAIRS)
                nc.gpsimd.dma_start(
                    out=rem[:],
                    in_=cc_out[ds(roff, 128), :].rearrange("p (m t) -> p m t", t=QT))
                nc.gpsimd.dma_start(out=own[:], in_=attn_T[:, :, ds(t_own[s], QT)])
                own_sb[s] = own
                rem_sb[s] = rem

            def proj_gen(s):
                """Output projection for the owned quarter of span s."""
                own, rem = own_sb[s], rem_sb[s]
                for i in range(2):
                    for e in range(2):
                        yield
                        psy = ps.tile([128, 512], F32, tag="m", bufs=2,
                                      name=f"psy{s}{i}{e}")
                        nc.tensor.matmul(
                            psy[:], ones_sb[0:1, 0:128],
                            bp_sb[:, e * 512:(e + 1) * 512],
                            start=True, stop=False)
                        for m in range(4):
                            nc.tensor.matmul(
                                psy[:], own[:, m, i * 128:(i + 1) * 128],
                                wpo_sb[:, m, e * 512:(e + 1) * 512],
                                start=False, stop=False)
                        for m in range(4):
                            nc.tensor.matmul(
                                psy[:], rem[:, m, i * 128:(i + 1) * 128],
                                wpx_sb[:, m, e * 512:(e + 1) * 512],
                                start=False, stop=(m == 3))
                        ysb = sb.tile([128, 512], F32, tag="ysb", bufs=2,
                                      name=f"ysb{s}{i}{e}")
                        nc.vector.tensor_copy(ysb[:], psy[:])
                        nc.sync.dma_start(
                            out=y[s, i * 128:(i + 1) * 128, e * 512:(e + 1) * 512],
                            in_=ysb[:])

            pumps = []
            proj_pumps = []

            def pump(allow_proj=False):
                while pumps:
                    try:
                        next(pumps[0])
                        return
                    except StopIteration:
                        pumps.pop(0)
                if allow_proj:
                    while proj_pumps:
                        try:
                            next(proj_pumps[0])
                            return
                        except StopIteration:
                            proj_pumps.pop(0)

            def drain(gen):
                for _ in gen:
                    pass

            # ---- chunk 0 QKV upfront, then span-major attention ----
            drain(qkv_gen(0))
            qkv_gens = {}
            for s in range(4):
                g_prev = qkv_gens.pop(s, None)
                if g_prev is not None and g_prev in pumps:
                    pumps.remove(g_prev)
                    drain(g_prev)
                if s < 3:
                    g_next = qkv_gen(s + 1)
                    qkv_gens[s + 1] = g_next
                    pumps.append(g_next)
                if s >= 1:
                    proj_pumps.append(proj_gen(s - 1))
                jmax = 4 * (s + 1)
                jm2 = jmax // 2
                for hp in range(4):
                    qspan = q_T[:, hp, s * SP:(s + 1) * SP]
                    pso = [ps.tile([65, 512], F32, tag="o", bufs=2,
                                   name=f"pso{s}{hp}{hh}")
                           for hh in range(2)]
                    prevP = [None, None]
                    # software pipeline: S/exp for unit jp, PV for jp-1
                    for jp in range(jm2 + 1):
                        # proj(s-1) only pumps from the third head-pair on,
                        # well after exchange(s-1) has landed (no PE
                        # head-of-line block on the pair collective)
                        pump(allow_proj=(hp >= 2))
                        curP = [None, None]
                        if jp < jm2:
                            pss = [None, None]
                            for u in range(2):
                                j = 2 * jp + u
                                off = max(0, (j - 4 * s) * 128)
                                for hh in range(2):
                                    mb = 64 * hh
                                    if u == 0:
                                        pss[hh] = ps.tile(
                                            [128, 1024], F32, tag="s", bufs=2,
                                            name=f"pss{s}{hp}{jp}{hh}")
                                    nc.tensor.matmul(
                                        pss[hh][:, u * 512 + off:(u + 1) * 512],
                                        k_T[mb:mb + 64, hp, j * 128:(j + 1) * 128],
                                        qspan[mb:mb + 64, off:512],
                                        start=True, stop=True)
                                if j >= 4 * s:  # diagonal: additive causal mask
                                    for hh in range(2):
                                        nc.vector.tensor_add(
                                            pss[hh][:, u * 512 + off:u * 512 + off + 128],
                                            pss[hh][:, u * 512 + off:u * 512 + off + 128],
                                            tri[:, 0:128])
                            for hh in range(2):
                                P = sb.tile([128, 1024], BF16, tag="P", bufs=4,
                                            name=f"P{s}{hp}{jp}{hh}")
                                for u in range(2):
                                    j = 2 * jp + u
                                    off = max(0, (j - 4 * s) * 128)
                                    # cols < off are never read downstream
                                    nc.scalar.activation(
                                        P[:, u * 512 + off:(u + 1) * 512],
                                        pss[hh][:, u * 512 + off:(u + 1) * 512],
                                        AF.Exp, scale=float(1.0 / 32.0))
                                curP[hh] = P
                        for hh in range(2):
                            if jp > 0:
                                h = 2 * hp + hh
                                Pp = prevP[hh]
                                for u in range(2):
                                    j = 2 * (jp - 1) + u
                                    off = max(0, (j - 4 * s) * 128)
                                    nc.tensor.matmul(
                                        pso[hh][:, off:512],
                                        v_aug[:, j, h * 65:h * 65 + 65],
                                        Pp[:, u * 512 + off:(u + 1) * 512],
                                        start=(j == 0), stop=(j == jmax - 1))
                        prevP = curP
                    # normalize: attn = oc[0:64] * bcast(1/oc[64])
                    ocs = []
                    for hh in range(2):
                        oc = sb.tile([65, 512], BF16, tag="oc", bufs=3,
                                     name=f"oc{s}{hp}{hh}")
                        with nc.allow_low_precision(reason="attn bf16"):
                            nc.vector.tensor_copy(oc[:], pso[hh][:])
                        ocs.append(oc)
                    for hh in range(2):
                        mb = 64 * hh
                        oc = ocs[hh]
                        psb2 = ps.tile([64, 512], F32, tag="o", bufs=2,
                                       name=f"psb2{s}{hp}{hh}")
                        nc.tensor.matmul(psb2[:], ones_sb[64:65, 0:64],
                                         oc[64:65, :], start=True, stop=True)
                        rcp = sb.tile([64, 512], F32, tag="rc", bufs=2,
                                      name=f"rcp{s}{hp}{hh}")
                        nc.vector.reciprocal_approx_fast(rcp[:], psb2[:])
                        with nc.allow_low_precision(reason="softmax recip"):
                            nc.vector.tensor_mul(
                                attn_T[mb:mb + 64, hp, s * SP:(s + 1) * SP],
                                oc[0:64, :], rcp[:])
                while proj_pumps:  # finish proj(s-1) before exchange(s)
                    try:
                        next(proj_pumps[0])
                    except StopIteration:
                        proj_pumps.pop(0)
                exchange(s)
            drain(proj_gen(3))

    nc.compile()
    return nc


def _get_nc():
    if "nc" not in _CACHE:
        _CACHE["nc"] = _build_nc()
    return _CACHE["nc"]


def _make_in_maps(x, Wq, Wk, Wv, Wp, bp):
    import ml_dtypes
    bf16 = ml_dtypes.bfloat16
    ones = np.ones((128, 128), bf16)
    in_maps = []
    for c in range(NCORES):
        b, g = c // 2, c % 2
        hsel = slice(g * HPC, (g + 1) * HPC)
        wq_c = np.ascontiguousarray(
            np.transpose(Wq[hsel], (1, 0, 2)).reshape(C, HPC * HS)).astype(bf16)
        wk_c = np.ascontiguousarray(
            np.transpose(Wk[hsel], (1, 0, 2)).reshape(C, HPC * HS)).astype(bf16)
        wv_c = np.ascontiguousarray(
            np.transpose(Wv[hsel], (1, 0, 2)).reshape(C, HPC * HS)).astype(bf16)
        in_maps.append({
            "xbT": np.ascontiguousarray(x[b].T).astype(bf16),
            "wq": wq_c, "wk": wk_c, "wv": wv_c,
            "wpo": np.ascontiguousarray(Wp[g * 512:(g + 1) * 512]).astype(bf16),
            "wpx": np.ascontiguousarray(Wp[(1 - g) * 512:(2 - g) * 512]).astype(bf16),
            "bpr": bp.reshape(1, C).astype(bf16),
            "onesd": ones,
        })
    return in_maps


def kernel(x, Wq, Wk, Wv, Wp, bp):
    from concourse.bass_utils import run_bass_kernel_spmd

    x = np.asarray(x, dtype=np.float32)
    Wq = np.asarray(Wq, dtype=np.float32)
    Wk = np.asarray(Wk, dtype=np.float32)
    Wv = np.asarray(Wv, dtype=np.float32)
    Wp = np.asarray(Wp, dtype=np.float32)
    bp = np.asarray(bp, dtype=np.float32)

    nc = _get_nc()
    in_maps = _make_in_maps(x, Wq, Wk, Wv, Wp, bp)
    res = run_bass_kernel_spmd(nc, in_maps, core_ids=list(range(NCORES)))
    _CACHE["last_results"] = res

    out = np.empty((B, T, C), np.float32)
    for c in range(NCORES):
        b, g = c // 2, c % 2
        yq = res.results[c]["y"]  # [4, QT, C]
        for s in range(4):
            t0 = s * SP + g * QT
            out[b, t0:t0 + QT, :] = yq[s]
    return out
